# revision 12
# baseline (speedup 1.0000x reference)
"""AttnBlock3D (GroupNorm + 1x1x1 QKV proj + per-frame spatial attention +
residual) for Trainium2, distributed over 8 NeuronCores.

Sharding: data-parallel over (frame, query-half). Core i handles frame
i // 2 and query pixels [(i % 2) * 2048, +2048) of that frame's 64x64 = 4096
pixels. Each core computes the full-frame K / V^T locally from its frame's
GroupNorm output (the [C, C] weights are replicated), so the only cross-core
traffic is one tiny 8-core AllReduce of the GroupNorm partial sums (the
GN statistics span all 4 frames).

The per-core input x_frame is pre-rotated on the host so the core's own 2048
query pixels always occupy columns 0:2048 -- the NEFF is pure SPMD with no
core-id-dependent control flow. Attention is permutation-invariant along the
key axis, so the rotated key order is harmless.

On-chip layout ([partition, free]):
  x_own  [128, 4(ct), 2048] f32   x for own query pixels (stats + residual)
  x_oth  [128, 4(ct), 2048] f32   rest of the frame (slot reused by vt)
  hn     [128, 4(ct), 4096] bf16  GroupNorm output
  q_all  [128, 4(ct), 2048] bf16  Q, channels on partitions
  k_all  [128, 4(ct), 4096] bf16  K, channels on partitions
  vt     [128, 32(kt), 512] bf16  V^T, key pixels on partitions
Scores are computed transposed (S^T[k, q]) so the kt-axis softmax sum comes
from a ones-vector matmul and the AV contraction has k on partitions.
"""

import numpy as np
import ml_dtypes

C = 512
T = 4
HW = 4096
HALF = HW // 2
P = 128
NCT = C // P        # 4 channel tiles
NQB = HALF // 512   # 4 query blocks per core
NKT = HW // P       # 32 key-pixel tiles
EPS = 1e-6
SCALE = float(C) ** -0.5
NCORES = 8

_CACHE = {}


def _build_bass():
    import concourse.bass as bass
    import concourse.bacc as bacc
    import concourse.mybir as mybir
    import concourse.tile as tile
    from contextlib import ExitStack

    f32 = mybir.dt.float32
    bf16 = mybir.dt.bfloat16
    Alu = mybir.AluOpType
    Act = mybir.ActivationFunctionType

    nc = bacc.Bacc(None, target_bir_lowering=False, num_devices=NCORES)

    x_in = nc.dram_tensor("x_frame", [C, HW], f32, kind="ExternalInput")
    wq_d = nc.dram_tensor("wqT", [C, C], bf16, kind="ExternalInput")
    wk_d = nc.dram_tensor("wkT", [C, C], bf16, kind="ExternalInput")
    wv_d = nc.dram_tensor("wvT", [C, C], bf16, kind="ExternalInput")
    wo_d = nc.dram_tensor("woT", [C, C], bf16, kind="ExternalInput")
    # rows: 0=bq 1=bk 2=bv 3=bo 4=gamma 5=beta
    params = nc.dram_tensor("params", [6, C], f32, kind="ExternalInput")
    out_d = nc.dram_tensor("out", [C, HALF], f32, kind="ExternalOutput")

    with tile.TileContext(nc) as tc, ExitStack() as ctx:
        big = ctx.enter_context(tc.tile_pool(name="big", bufs=1))
        ovl = ctx.enter_context(tc.tile_pool(name="ovl", bufs=1))
        const = ctx.enter_context(tc.tile_pool(name="const", bufs=1))
        small = ctx.enter_context(tc.tile_pool(name="small", bufs=1))
        stats_p = ctx.enter_context(tc.tile_pool(name="stats_p", bufs=2))
        expp = ctx.enter_context(tc.tile_pool(name="expp", bufs=3))
        oavp = ctx.enter_context(tc.tile_pool(name="oavp", bufs=5))
        rbp = ctx.enter_context(tc.tile_pool(name="rbp", bufs=2))
        foutp = ctx.enter_context(tc.tile_pool(name="foutp", bufs=3))
        psA = ctx.enter_context(tc.tile_pool(name="psA", bufs=3, space="PSUM"))
        psAV = ctx.enter_context(tc.tile_pool(name="psAV", bufs=4, space="PSUM"))
        psS = ctx.enter_context(tc.tile_pool(name="psS", bufs=1, space="PSUM"))
        drp = ctx.enter_context(tc.tile_pool(name="drp", bufs=1, space="DRAM"))

        # ---- constants / params -------------------------------------------
        wq_sb = const.tile([P, NCT, C], bf16)
        wk_sb = const.tile([P, NCT, C], bf16)
        wv_sb = const.tile([P, NCT, C], bf16)
        wo_sb = const.tile([P, NCT, C], bf16)
        for w_sb, w_d in ((wq_sb, wq_d), (wk_sb, wk_d), (wv_sb, wv_d),
                          (wo_sb, wo_d)):
            for ci in range(NCT):
                nc.sync.dma_start(out=w_sb[:, ci, :],
                                  in_=w_d[ci * P:(ci + 1) * P, :])

        bias_q = const.tile([P, NCT], f32)
        bias_k = const.tile([P, NCT], f32)
        bias_o = const.tile([P, NCT], f32)
        gb = const.tile([P, NCT, 2], f32)
        for t_sb, row in ((bias_q, 0), (bias_k, 1), (bias_o, 3)):
            nc.sync.dma_start(
                out=t_sb, in_=params[row].rearrange("(ct p) -> p ct", p=P))
        nc.sync.dma_start(
            out=gb[:, :, 0], in_=params[4].rearrange("(ct p) -> p ct", p=P))
        nc.sync.dma_start(
            out=gb[:, :, 1], in_=params[5].rearrange("(ct p) -> p ct", p=P))
        # bv broadcast across partitions: [128, 512]
        bvb = const.tile([P, C], f32)
        bv_row = params[2]
        nc.gpsimd.dma_start(
            out=bvb,
            in_=bass.AP(tensor=bv_row.tensor, offset=bv_row.offset,
                        ap=[[0, P], [1, C]]))

        ones_col = const.tile([P, 1], bf16)
        nc.vector.memset(ones_col, 1.0)
        ones_row = const.tile([1, P], f32)
        nc.vector.memset(ones_row, 1.0)
        sel_np = np.zeros((P, 8), np.float32)
        for j in range(8):
            sel_np[j * 16:(j + 1) * 16, j] = 1.0
        sel_d = nc.inline_tensor(sel_np, name="sel_const")
        sel_stage = const.tile([P, 8], f32)
        nc.sync.dma_start(out=sel_stage, in_=sel_d[:, :])
        # staged through DVE so the stats matmul waits on a single sem domain
        # (walrus can't encode two waits on an inline-LDWEIGHTS matmul)
        sel = const.tile([P, 8], f32)
        nc.vector.tensor_copy(out=sel, in_=sel_stage)
        eps8 = const.tile([8, 1], f32)
        nc.vector.memset(eps8, EPS)

        # ---- load x -------------------------------------------------------
        x_own = big.tile([P, NCT, HALF], f32)
        x_oth = ovl.tile([P, NCT, HALF], f32, tag="ov")
        for ct in range(NCT):
            nc.sync.dma_start(out=x_own[:, ct, :],
                              in_=x_in[ct * P:(ct + 1) * P, 0:HALF])
        for ct in range(NCT):
            nc.sync.dma_start(out=x_oth[:, ct, :],
                              in_=x_in[ct * P:(ct + 1) * P, HALF:HW])

        # ---- GroupNorm partial stats over own pixels + AllReduce ----------
        with nc.named_scope("gn_stats"):
            mvall = small.tile([P, NCT, 2], f32)
            for ct in range(NCT):
                st6 = stats_p.tile([P, 4, 6], f32, tag="st6", name=f"st6_{ct}")
                for j in range(4):
                    nc.vector.bn_stats(
                        out=st6[:, j, :],
                        in_=x_own[:, ct, j * 512:(j + 1) * 512])
                nc.vector.bn_aggr(out=mvall[:, ct, :], in_=st6)
            # su columns 0:4 = per-channel mean, 4:8 = per-channel E[x^2]
            su = small.tile([P, 8], f32)
            nc.vector.tensor_copy(out=su[:, 0:4], in_=mvall[:, :, 0])
            nc.vector.tensor_mul(out=su[:, 4:8], in0=mvall[:, :, 0],
                                 in1=mvall[:, :, 0])
            nc.vector.tensor_add(out=su[:, 4:8], in0=su[:, 4:8],
                                 in1=mvall[:, :, 1])
            # reduce the 16 channels of each group across partitions
            st_ps = psA.tile([8, 8], f32, tag="st")
            nc.tensor.matmul(st_ps, lhsT=sel, rhs=su, start=True, stop=True)
            cc_in_sb = small.tile([8, 8], f32)
            nc.vector.tensor_copy(out=cc_in_sb, in_=st_ps)
            cc_in = drp.tile([8, 8], f32)
            cc_out = drp.tile([8, 8], f32)
            nc.gpsimd.dma_start(out=cc_in, in_=cc_in_sb)
            nc.gpsimd.collective_compute(
                "AllReduce", mybir.AluOpType.add,
                replica_groups=[list(range(NCORES))],
                ins=[cc_in.opt()], outs=[cc_out.opt()])
            allst = small.tile([8, 8], f32)
            nc.gpsimd.dma_start(out=allst, in_=cc_out)

            # group mean / rstd  (allst[j, ct] = S_g/2048, N_g = 262144)
            musd8 = small.tile([8, NCT, 2], f32)
            tmp8 = small.tile([8, NCT], f32)
            tmpb8 = small.tile([8, NCT], f32)
            sq8 = small.tile([8, NCT], f32)
            nc.vector.tensor_scalar_mul(musd8[:, :, 0], allst[:, 0:4],
                                        1.0 / 128.0)
            nc.vector.tensor_scalar_mul(tmp8, allst[:, 4:8], 1.0 / 128.0)
            nc.vector.tensor_mul(out=tmpb8, in0=musd8[:, :, 0],
                                 in1=musd8[:, :, 0])
            nc.vector.tensor_sub(out=tmp8, in0=tmp8, in1=tmpb8)
            nc.scalar.activation(out=sq8, in_=tmp8, func=Act.Sqrt, bias=eps8)
            nc.vector.reciprocal(out=musd8[:, :, 1], in_=sq8)

            # broadcast group stats to per-channel [128, ct, 2] via DRAM
            stat8 = drp.tile([8, NCT, 2], f32)
            nc.gpsimd.dma_start(out=stat8, in_=musd8)
            musd = small.tile([P, NCT, 2], f32)
            nc.gpsimd.dma_start(
                out=musd,
                in_=bass.AP(tensor=stat8.tensor, offset=stat8.offset,
                            ap=[[8, 8], [0, 16], [2, NCT], [1, 2]]))
            # s1 = rstd * gamma ; s2 = beta - mu * s1
            s1 = small.tile([P, NCT], f32)
            s2 = small.tile([P, NCT], f32)
            s2t = small.tile([P, NCT], f32)
            nc.vector.tensor_mul(out=s1, in0=musd[:, :, 1], in1=gb[:, :, 0])
            nc.vector.tensor_mul(out=s2t, in0=musd[:, :, 0], in1=s1)
            nc.vector.tensor_sub(out=s2, in0=gb[:, :, 1], in1=s2t)

        # ---- GN apply: hn = x * s1 + s2 (bf16) ----------------------------
        hn = big.tile([P, NCT, HW], bf16)
        with nc.named_scope("gn_apply"):
            for ct in range(NCT):
                nc.vector.tensor_scalar(
                    out=hn[:, ct, 0:HALF], in0=x_own[:, ct, :],
                    scalar1=s1[:, ct:ct + 1], scalar2=s2[:, ct:ct + 1],
                    op0=Alu.mult, op1=Alu.add)
                nc.vector.tensor_scalar(
                    out=hn[:, ct, HALF:HW], in0=x_oth[:, ct, :],
                    scalar1=s1[:, ct:ct + 1], scalar2=s2[:, ct:ct + 1],
                    op0=Alu.mult, op1=Alu.add)

        # ---- projections --------------------------------------------------
        k_all = big.tile([P, NCT, HW], bf16)
        q_all = big.tile([P, NCT, HALF], bf16)
        vt = ovl.tile([P, NKT, C], bf16, tag="ov")

        with nc.named_scope("proj_k"):
            for co in range(NCT):
                for nk in range(HW // 512):
                    pk = psA.tile([P, 512], f32, tag="st",
                                  name=f"pk_{co}_{nk}")
                    for ci in range(NCT):
                        nc.tensor.matmul(
                            pk, lhsT=wk_sb[:, ci, co * P:(co + 1) * P],
                            rhs=hn[:, ci, nk * 512:(nk + 1) * 512],
                            start=(ci == 0), stop=(ci == NCT - 1))
                    nc.vector.tensor_scalar_add(
                        out=k_all[:, co, nk * 512:(nk + 1) * 512], in0=pk,
                        scalar1=bias_k[:, co:co + 1])
        with nc.named_scope("proj_v"):
            for kt in range(NKT):
                pv = psA.tile([P, C], f32, tag="st", name=f"pv_{kt}")
                for ci in range(NCT):
                    nc.tensor.matmul(
                        pv, lhsT=hn[:, ci, kt * P:(kt + 1) * P],
                        rhs=wv_sb[:, ci, :],
                        start=(ci == 0), stop=(ci == NCT - 1))
                nc.vector.tensor_add(out=vt[:, kt, :], in0=pv, in1=bvb)
        with nc.named_scope("proj_q"):
            for co in range(NCT):
                for nq in range(NQB):
                    pq = psA.tile([P, 512], f32, tag="st",
                                  name=f"pq_{co}_{nq}")
                    for ci in range(NCT):
                        nc.tensor.matmul(
                            pq, lhsT=wq_sb[:, ci, co * P:(co + 1) * P],
                            rhs=hn[:, ci, nq * 512:(nq + 1) * 512],
                            start=(ci == 0), stop=(ci == NCT - 1))
                    nc.vector.tensor_scalar_add(
                        out=q_all[:, co, nq * 512:(nq + 1) * 512], in0=pq,
                        scalar1=bias_q[:, co:co + 1])

        # ---- attention ----------------------------------------------------
        # Software-pipelined: S^T matmuls for kt run one step ahead of the
        # exp-consuming (sums + AV) matmuls so the PE never waits on ACT.
        for qb in range(NQB):
            with nc.named_scope(f"attn_qb{qb}"):
                qsl = slice(qb * 512, (qb + 1) * 512)
                sums_ps = psS.tile([1, 512], f32, tag="sums",
                                   name=f"sums_{qb}")
                av_ps = [psAV.tile([P, 512], f32, tag="av",
                                   name=f"av_{qb}_{co}") for co in range(NCT)]
                ex_tiles = [None] * NKT

                def consume(kt):
                    ex = ex_tiles[kt]
                    nc.tensor.matmul(sums_ps, lhsT=ones_col, rhs=ex,
                                     start=(kt == 0), stop=(kt == NKT - 1))
                    for co in range(NCT):
                        nc.tensor.matmul(
                            av_ps[co],
                            lhsT=vt[:, kt, co * P:(co + 1) * P], rhs=ex,
                            start=(kt == 0), stop=(kt == NKT - 1))

                for kt in range(NKT):
                    st_ps = psA.tile([P, 512], f32, tag="st",
                                     name=f"st_{qb}_{kt}")
                    for ci in range(NCT):
                        nc.tensor.matmul(
                            st_ps, lhsT=k_all[:, ci, kt * P:(kt + 1) * P],
                            rhs=q_all[:, ci, qsl],
                            start=(ci == 0), stop=(ci == NCT - 1))
                    ex = expp.tile([P, 512], bf16, tag="ex", name=f"ex_{qb}_{kt}")
                    nc.scalar.activation(out=ex, in_=st_ps, func=Act.Exp,
                                         scale=SCALE)
                    ex_tiles[kt] = ex
                    if kt > 0:
                        consume(kt - 1)
                consume(NKT - 1)

                # 1/sum, broadcast across partitions via a rank-1 matmul
                rsum = rbp.tile([1, 512], f32, tag="rsum", name=f"rsum_{qb}")
                nc.vector.reciprocal(out=rsum, in_=sums_ps)
                rb_ps = psA.tile([P, 512], f32, tag="st", name=f"rb_{qb}")
                nc.tensor.matmul(rb_ps, lhsT=ones_row, rhs=rsum,
                                 start=True, stop=True)
                rb_sb = rbp.tile([P, 512], f32, tag="rb_sb",
                                 name=f"rb_sb_{qb}")
                nc.vector.tensor_copy(out=rb_sb, in_=rb_ps)
                oav = [oavp.tile([P, 512], bf16, tag="oav", name=f"oav_{qb}_{co}")
                       for co in range(NCT)]
                for co in range(NCT):
                    nc.vector.tensor_mul(out=oav[co], in0=av_ps[co],
                                         in1=rb_sb)
                # output projection + bias + residual
                for co in range(NCT):
                    fp_ps = psAV.tile([P, 512], f32, tag="av",
                                      name=f"fp_{qb}_{co}")
                    for ci in range(NCT):
                        nc.tensor.matmul(
                            fp_ps, lhsT=wo_sb[:, ci, co * P:(co + 1) * P],
                            rhs=oav[ci], start=(ci == 0),
                            stop=(ci == NCT - 1))
                    fout = foutp.tile([P, 512], f32, tag="fout", name=f"fout_{qb}_{co}")
                    nc.vector.tensor_scalar_add(out=fout, in0=fp_ps,
                                                scalar1=bias_o[:, co:co + 1])
                    nc.vector.tensor_add(out=fout, in0=fout,
                                         in1=x_own[:, co, qsl])
                    nc.sync.dma_start(out=out_d[co * P:(co + 1) * P, qsl],
                                      in_=fout)

    nc.finalize()
    return nc


def _get_nc():
    if "nc" not in _CACHE:
        _CACHE["nc"] = _build_bass()
    return _CACHE["nc"]


def _prepare_in_maps(inputs):
    x = np.asarray(inputs["x"], dtype=np.float32)
    b, c, t, h, w = x.shape
    assert (b, c, t, h * w) == (1, C, T, HW)

    wts = {}
    for name, key in (("wqT", "wq"), ("wkT", "wk"), ("wvT", "wv"),
                      ("woT", "wo")):
        wts[name] = np.ascontiguousarray(
            np.asarray(inputs[key], dtype=np.float32).T
        ).astype(ml_dtypes.bfloat16)
    params = np.stack([
        np.asarray(inputs["bq"]), np.asarray(inputs["bk"]),
        np.asarray(inputs["bv"]), np.asarray(inputs["bo"]),
        np.asarray(inputs["gamma"]), np.asarray(inputs["beta"]),
    ]).astype(np.float32)

    xr = np.ascontiguousarray(x.reshape(C, T, HW))
    in_maps = []
    for core in range(NCORES):
        f, half = core // 2, core % 2
        own = xr[:, f, half * HALF:(half + 1) * HALF]
        oth = xr[:, f, (1 - half) * HALF:(2 - half) * HALF]
        x_frame = np.ascontiguousarray(np.concatenate([own, oth], axis=1))
        in_maps.append({"x_frame": x_frame, "params": params, **wts})
    return in_maps


def _assemble_out(per_core_outs):
    out = np.empty((C, T, HW), np.float32)
    for core in range(NCORES):
        f, half = core // 2, core % 2
        out[:, f, half * HALF:(half + 1) * HALF] = per_core_outs[core]
    return out.reshape(1, C, T, 64, 64)


def kernel(**inputs):
    import os
    from concourse.bass_utils import run_bass_kernel_spmd

    in_maps = _prepare_in_maps(inputs)
    nc = _get_nc()
    trace = os.environ.get("KBENCH_TRACE") == "1"
    if trace:
        try:
            from antenv.axon_hooks import get_axon_ntff_profile_hook  # noqa: F401
        except ImportError:
            trace = False
    res = run_bass_kernel_spmd(nc, in_maps, core_ids=list(range(NCORES)),
                               trace=trace)
    _CACHE["last_res"] = res
    return _assemble_out([r["out"] for r in res.results])


# revision 14
# speedup vs baseline: 1.0415x; 1.0415x over previous
"""AttnBlock3D for Trainium2, 8 NeuronCores. v3: fp8 DoubleRow everywhere.

Sharding: core i = (frame i//2, query-half i%2); full-frame K/V computed
locally; one tiny AllReduce exchanges GroupNorm partial sums.

v3 trick: GroupNorm is folded into the projections. With per-channel
s1 = rstd*gamma, s2 = beta - mu*s1 (known only after the stats AllReduce):
    q = Wq(s1*x + s2) + bq = (Wq*diag(s1)) x + (bq + Wq s2)
so the kernel casts x to fp8 while the AllReduce is in flight (no stats
dep), then scales the weights by s1 (tiny) and runs all QKV projections as
fp8 DoubleRow matmuls straight off fp8(x) -- the 16MB GroupNorm-apply pass
disappears. The residual path keeps x in fp32.

Attention is fp8 DoubleRow with scores computed transposed (S^T[k,q]);
softmax skips max-subtraction (|S| <= ~2 by construction); the k-axis sum
comes from a ones-vector matmul riding the same fp8 pipeline.
"""

import numpy as np
import ml_dtypes

C = 512
T = 4
HW = 4096
HALF = HW // 2
P = 128
NCT = C // P        # 4 channel tiles
NQB = HALF // 512   # 4 query blocks per core
NKT = HW // P       # 32 key-pixel tiles
EPS = 1e-6
SCALE = float(C) ** -0.5
NCORES = 8

_CACHE = {}


def _build_bass():
    import concourse.bass as bass
    import concourse.bacc as bacc
    import concourse.mybir as mybir
    import concourse.tile as tile
    from contextlib import ExitStack

    f32 = mybir.dt.float32
    bf16 = mybir.dt.bfloat16
    fp8 = mybir.dt.float8e4
    Alu = mybir.AluOpType
    Act = mybir.ActivationFunctionType
    DR = mybir.MatmulPerfMode.DoubleRow

    nc = bacc.Bacc(None, target_bir_lowering=False, num_devices=NCORES)

    x_in = nc.dram_tensor("x_frame", [C, HW], f32, kind="ExternalInput")
    wq_d = nc.dram_tensor("wqT", [C, C], bf16, kind="ExternalInput")
    wk_d = nc.dram_tensor("wkT", [C, C], bf16, kind="ExternalInput")
    wv_d = nc.dram_tensor("wvT", [C, C], bf16, kind="ExternalInput")
    wo_d = nc.dram_tensor("woT", [C, C], bf16, kind="ExternalInput")
    # rows: 0=bq 1=bk 2=bv 3=bo 4=gamma 5=beta
    params = nc.dram_tensor("params", [6, C], f32, kind="ExternalInput")
    out_d = nc.dram_tensor("out", [C, HALF], f32, kind="ExternalOutput")

    with tile.TileContext(nc) as tc, ExitStack() as ctx:
        big = ctx.enter_context(tc.tile_pool(name="big", bufs=1))
        xoth = ctx.enter_context(tc.tile_pool(name="xoth", bufs=2))
        const = ctx.enter_context(tc.tile_pool(name="const", bufs=1))
        small = ctx.enter_context(tc.tile_pool(name="small", bufs=1))
        stats_p = ctx.enter_context(tc.tile_pool(name="stats_p", bufs=2))
        expp = ctx.enter_context(tc.tile_pool(name="expp", bufs=3))
        oavp = ctx.enter_context(tc.tile_pool(name="oavp", bufs=5))
        rbp = ctx.enter_context(tc.tile_pool(name="rbp", bufs=2))
        foutp = ctx.enter_context(tc.tile_pool(name="foutp", bufs=3))
        psA = ctx.enter_context(tc.tile_pool(name="psA", bufs=3, space="PSUM"))
        psS = ctx.enter_context(tc.tile_pool(name="psS", bufs=1, space="PSUM"))
        psAV = ctx.enter_context(tc.tile_pool(name="psAV", bufs=4,
                                              space="PSUM"))
        drp = ctx.enter_context(tc.tile_pool(name="drp", bufs=1, space="DRAM"))

        # ---- weights / params ---------------------------------------------
        bias_q = const.tile([P, NCT], f32)
        bias_k = const.tile([P, NCT], f32)
        bias_o = const.tile([P, NCT], f32)
        gb = const.tile([P, NCT, 2], f32)
        for t_sb, row in ((bias_q, 0), (bias_k, 1), (bias_o, 3)):
            nc.sync.dma_start(
                out=t_sb, in_=params[row].rearrange("(ct p) -> p ct", p=P))
        nc.sync.dma_start(
            out=gb[:, :, 0], in_=params[4].rearrange("(ct p) -> p ct", p=P))
        nc.sync.dma_start(
            out=gb[:, :, 1], in_=params[5].rearrange("(ct p) -> p ct", p=P))
        bvb = const.tile([P, C], f32)
        bv_row = params[2]
        nc.gpsimd.dma_start(
            out=bvb,
            in_=bass.AP(tensor=bv_row.tensor, offset=bv_row.offset,
                        ap=[[0, P], [1, C]]))

        ones_col_t = const.tile([P, 2, 16], fp8)
        nc.vector.memset(ones_col_t, 1.0)
        ones_col = ones_col_t[:, :, 0:1]
        ones_row = const.tile([1, P], f32)
        nc.vector.memset(ones_row, 1.0)
        sel_np = np.zeros((P, 8), np.float32)
        for j in range(8):
            sel_np[j * 16:(j + 1) * 16, j] = 1.0
        sel_d = nc.inline_tensor(sel_np, name="sel_const")
        sel_stage = const.tile([P, 8], f32)
        nc.sync.dma_start(out=sel_stage, in_=sel_d[:, :])
        sel = const.tile([P, 8], f32)
        nc.vector.tensor_copy(out=sel, in_=sel_stage)
        eps8 = const.tile([8, 1], f32)
        nc.vector.memset(eps8, EPS)

        # ---- x loads first (the stats chain is the critical path) ---------
        x_own = big.tile([P, NCT, HALF], f32)
        for ct in range(NCT):
            for hh in range(2):
                nc.sync.dma_start(
                    out=x_own[:, ct, hh * 1024:(hh + 1) * 1024],
                    in_=x_in[ct * P:(ct + 1) * P, hh * 1024:(hh + 1) * 1024])
        ot_tiles = []
        for ct in range(NCT):
            ot = xoth.tile([P, HALF], f32, tag="ot", name=f"ot_{ct}")
            nc.sync.dma_start(out=ot,
                              in_=x_in[ct * P:(ct + 1) * P, HALF:HW])
            ot_tiles.append(ot)

        wq_sb = const.tile([P, NCT, C], bf16)
        wk_sb = const.tile([P, NCT, C], bf16)
        wv_sb = const.tile([P, NCT, C], bf16)
        wo_sb = const.tile([P, NCT, C], bf16)
        for w_sb, w_d in ((wq_sb, wq_d), (wk_sb, wk_d), (wv_sb, wv_d),
                          (wo_sb, wo_d)):
            for ci in range(NCT):
                nc.sync.dma_start(out=w_sb[:, ci, :],
                                  in_=w_d[ci * P:(ci + 1) * P, :])

        # ---- GroupNorm partial stats over own pixels + AllReduce ----------
        with nc.named_scope("gn_stats"):
            mvall = small.tile([P, NCT, 2], f32)
            for ct in range(NCT):
                st6 = stats_p.tile([P, 4, 6], f32, tag="st6",
                                   name=f"st6_{ct}")
                for j in range(4):
                    nc.vector.bn_stats(
                        out=st6[:, j, :],
                        in_=x_own[:, ct, j * 512:(j + 1) * 512])
                nc.vector.bn_aggr(out=mvall[:, ct, :], in_=st6)
            su = small.tile([P, 8], f32)
            nc.vector.tensor_copy(out=su[:, 0:4], in_=mvall[:, :, 0])
            nc.vector.tensor_mul(out=su[:, 4:8], in0=mvall[:, :, 0],
                                 in1=mvall[:, :, 0])
            nc.vector.tensor_add(out=su[:, 4:8], in0=su[:, 4:8],
                                 in1=mvall[:, :, 1])
            st_ps = psA.tile([8, 8], f32, tag="st")
            nc.tensor.matmul(st_ps, lhsT=sel, rhs=su, start=True, stop=True)
            cc_in_sb = small.tile([8, 8], f32)
            nc.vector.tensor_copy(out=cc_in_sb, in_=st_ps)
            cc_in = drp.tile([8, 8], f32)
            cc_out = drp.tile([8, 8], f32)
            nc.gpsimd.dma_start(out=cc_in, in_=cc_in_sb)
            nc.gpsimd.collective_compute(
                "AllReduce", mybir.AluOpType.add,
                replica_groups=[list(range(NCORES))],
                ins=[cc_in.opt()], outs=[cc_out.opt()])
            allst = small.tile([8, 8], f32)
            nc.gpsimd.dma_start(out=allst, in_=cc_out)

        # ---- x -> fp8 casts (no stats dependency; overlaps AllReduce) -----
        x_f8 = big.tile([P, NCT, HW], fp8)
        with nc.named_scope("x_cast"):
            for ct in range(NCT):
                nc.vector.tensor_copy(out=x_f8[:, ct, 0:HALF],
                                      in_=x_own[:, ct, :])
            for ct in range(NCT):
                nc.vector.tensor_copy(out=x_f8[:, ct, HALF:HW],
                                      in_=ot_tiles[ct])

        # ---- group stats -> s1/s2 -> scaled fp8 weights + bias folds ------
        with nc.named_scope("gn_fold"):
            musd8 = small.tile([8, NCT, 2], f32)
            tmp8 = small.tile([8, NCT], f32)
            tmpb8 = small.tile([8, NCT], f32)
            sq8 = small.tile([8, NCT], f32)
            nc.vector.tensor_scalar_mul(musd8[:, :, 0], allst[:, 0:4],
                                        1.0 / 128.0)
            nc.vector.tensor_scalar_mul(tmp8, allst[:, 4:8], 1.0 / 128.0)
            nc.vector.tensor_mul(out=tmpb8, in0=musd8[:, :, 0],
                                 in1=musd8[:, :, 0])
            nc.vector.tensor_sub(out=tmp8, in0=tmp8, in1=tmpb8)
            nc.scalar.activation(out=sq8, in_=tmp8, func=Act.Sqrt, bias=eps8)
            nc.vector.reciprocal(out=musd8[:, :, 1], in_=sq8)

            stat8 = drp.tile([8, NCT, 2], f32)
            nc.gpsimd.dma_start(out=stat8, in_=musd8)
            musd = small.tile([P, NCT, 2], f32)
            nc.gpsimd.dma_start(
                out=musd,
                in_=bass.AP(tensor=stat8.tensor, offset=stat8.offset,
                            ap=[[8, 8], [0, 16], [2, NCT], [1, 2]]))
            s1 = small.tile([P, NCT], f32)
            s2 = small.tile([P, NCT], f32)
            s2t = small.tile([P, NCT], f32)
            s2b = small.tile([P, NCT], bf16)
            nc.vector.tensor_mul(out=s1, in0=musd[:, :, 1], in1=gb[:, :, 0])
            nc.vector.tensor_mul(out=s2t, in0=musd[:, :, 0], in1=s1)
            nc.vector.tensor_sub(out=s2, in0=gb[:, :, 1], in1=s2t)
            nc.vector.tensor_copy(out=s2b, in_=s2)

            # W' = W * diag(s1), cast to fp8 (per input-channel partition)
            wq_f8 = const.tile([P, NCT, C], fp8)
            wk_f8 = const.tile([P, NCT, C], fp8)
            wv_f8 = const.tile([P, NCT, C], fp8)
            for w_f8, w_sb in ((wq_f8, wq_sb), (wk_f8, wk_sb),
                               (wv_f8, wv_sb)):
                for ci in range(NCT):
                    nc.vector.tensor_scalar_mul(
                        out=w_f8[:, ci, :], in0=w_sb[:, ci, :],
                        scalar1=s1[:, ci:ci + 1])

            # bias folds: b' = b + W s2  (fold row [1, C] via matmul)
            bias_q2 = const.tile([P, NCT], f32)
            bias_k2 = const.tile([P, NCT], f32)
            for w_sb, bias_in, bias_out, tagn in (
                    (wq_sb, bias_q, bias_q2, "fq"),
                    (wk_sb, bias_k, bias_k2, "fk")):
                fold_ps = psA.tile([1, C], f32, tag="st", name=f"fps_{tagn}")
                for ci in range(NCT):
                    nc.tensor.matmul(fold_ps, lhsT=s2b[:, ci:ci + 1],
                                     rhs=w_sb[:, ci, :], start=(ci == 0),
                                     stop=(ci == NCT - 1))
                fold_sb = small.tile([1, C], f32, name=f"fsb_{tagn}")
                nc.vector.tensor_copy(out=fold_sb, in_=fold_ps)
                fold_d = drp.tile([C], f32, name=f"fd_{tagn}")
                nc.gpsimd.dma_start(out=fold_d, in_=fold_sb)
                foldT = small.tile([P, NCT], f32, name=f"ft_{tagn}")
                nc.gpsimd.dma_start(
                    out=foldT,
                    in_=bass.AP(tensor=fold_d.tensor, offset=fold_d.offset,
                                ap=[[1, P], [P, NCT]]))
                nc.vector.tensor_add(out=bias_out, in0=bias_in, in1=foldT)
            # v fold stays a row; broadcast to [128, C] and add into bvb
            fold_ps_v = psA.tile([1, C], f32, tag="st", name="fps_v")
            for ci in range(NCT):
                nc.tensor.matmul(fold_ps_v, lhsT=s2b[:, ci:ci + 1],
                                 rhs=wv_sb[:, ci, :], start=(ci == 0),
                                 stop=(ci == NCT - 1))
            fold_sb_v = small.tile([1, C], f32)
            nc.vector.tensor_copy(out=fold_sb_v, in_=fold_ps_v)
            fold_bc_ps = psA.tile([P, C], f32, tag="st", name="fbc_v")
            nc.tensor.matmul(fold_bc_ps, lhsT=ones_row, rhs=fold_sb_v,
                             start=True, stop=True)
            bvb2 = const.tile([P, C], f32)
            nc.vector.tensor_add(out=bvb2, in0=bvb, in1=fold_bc_ps)

        # ---- projections: fp8 DoubleRow off fp8(x) ------------------------
        k_all = big.tile([P, NCT, HW], fp8)
        q_all = big.tile([P, NCT, HALF], fp8)
        vt = big.tile([P, NKT, C], fp8)

        with nc.named_scope("proj"):
            # interleave K / Q / V tiles; K copies drain on ACT, Q/V on DVE,
            # psum tiles alternate pools so both copy engines run in parallel
            tiles = []
            for co in range(NCT):
                for nk in range(HW // 512):
                    tiles.append(("k", co, nk))
            for co in range(NCT):
                for nq in range(NQB):
                    tiles.append(("q", co, nq))
            for kt in range(NKT):
                tiles.append(("v", kt, 0))
            # round-robin: k, v, k, v, q, ... keep PE dense on both pools
            order = []
            ks = [t for t in tiles if t[0] == "k"]
            vs = [t for t in tiles if t[0] == "v"]
            qs = [t for t in tiles if t[0] == "q"]
            n = max(len(ks), len(vs), len(qs))
            for i in range(n):
                if i < len(ks):
                    order.append(ks[i])
                if i < len(vs):
                    order.append(vs[i])
                if i < len(qs):
                    order.append(qs[i])
            for kind, a, b in order:
                if kind == "k":
                    co, nk = a, b
                    pk = psA.tile([P, 512], f32, tag="st",
                                  name=f"pk_{co}_{nk}")
                    for u in range(2):
                        nc.tensor.matmul(
                            pk,
                            lhsT=wk_f8[:, 2 * u:2 * u + 2,
                                       co * P:(co + 1) * P],
                            rhs=x_f8[:, 2 * u:2 * u + 2,
                                     nk * 512:(nk + 1) * 512],
                            start=(u == 0), stop=(u == 1), perf_mode=DR)
                    nc.scalar.activation(
                        out=k_all[:, co, nk * 512:(nk + 1) * 512], in_=pk,
                        func=Act.Identity, bias=bias_k2[:, co:co + 1])
                elif kind == "v":
                    kt = a
                    pv = psAV.tile([P, C], f32, tag="av", name=f"pv_{kt}")
                    for u in range(2):
                        nc.tensor.matmul(
                            pv,
                            lhsT=x_f8[:, 2 * u:2 * u + 2,
                                      kt * P:(kt + 1) * P],
                            rhs=wv_f8[:, 2 * u:2 * u + 2, :],
                            start=(u == 0), stop=(u == 1), perf_mode=DR)
                    nc.vector.tensor_add(out=vt[:, kt, :], in0=pv, in1=bvb2)
                else:
                    co, nq = a, b
                    pq = psA.tile([P, 512], f32, tag="st",
                                  name=f"pq_{co}_{nq}")
                    for u in range(2):
                        nc.tensor.matmul(
                            pq,
                            lhsT=wq_f8[:, 2 * u:2 * u + 2,
                                       co * P:(co + 1) * P],
                            rhs=x_f8[:, 2 * u:2 * u + 2,
                                     nq * 512:(nq + 1) * 512],
                            start=(u == 0), stop=(u == 1), perf_mode=DR)
                    nc.vector.tensor_scalar_add(
                        out=q_all[:, co, nq * 512:(nq + 1) * 512], in0=pq,
                        scalar1=bias_q2[:, co:co + 1])

        # ---- attention: fp8 DoubleRow, S^T layout -------------------------
        NKP = NKT // 2
        for qb in range(NQB):
            with nc.named_scope(f"attn_qb{qb}"):
                qsl = slice(qb * 512, (qb + 1) * 512)
                sums_ps = psS.tile([1, 512], f32, tag="sums",
                                   name=f"sums_{qb}")
                av_ps = [psAV.tile([P, 512], f32, tag="av",
                                   name=f"av_{qb}_{co}") for co in range(NCT)]
                ex_tiles = [None] * NKP

                def consume(kp):
                    ex2 = ex_tiles[kp]
                    nc.tensor.matmul(sums_ps, lhsT=ones_col, rhs=ex2,
                                     start=(kp == 0), stop=(kp == NKP - 1),
                                     perf_mode=DR)
                    for co in range(NCT):
                        nc.tensor.matmul(
                            av_ps[co],
                            lhsT=vt[:, 2 * kp:2 * kp + 2,
                                    co * P:(co + 1) * P],
                            rhs=ex2, start=(kp == 0), stop=(kp == NKP - 1),
                            perf_mode=DR)

                for kp in range(NKP):
                    ex2 = expp.tile([P, 2, 512], fp8, tag="ex",
                                    name=f"ex_{qb}_{kp}")
                    for half in range(2):
                        kt = 2 * kp + half
                        st_ps = psA.tile([P, 512], f32, tag="st",
                                         name=f"st_{qb}_{kt}")
                        for u in range(2):
                            nc.tensor.matmul(
                                st_ps,
                                lhsT=k_all[:, 2 * u:2 * u + 2,
                                           kt * P:(kt + 1) * P],
                                rhs=q_all[:, 2 * u:2 * u + 2, qsl],
                                start=(u == 0), stop=(u == 1), perf_mode=DR)
                        nc.scalar.activation(out=ex2[:, half, :], in_=st_ps,
                                             func=Act.Exp, scale=SCALE)
                    ex_tiles[kp] = ex2
                    if kp > 1:
                        consume(kp - 2)
                consume(NKP - 2)
                consume(NKP - 1)

                rsum = rbp.tile([1, 512], f32, tag="rsum", name=f"rsum_{qb}")
                nc.vector.reciprocal(out=rsum, in_=sums_ps)
                rb_ps = psA.tile([P, 512], f32, tag="st", name=f"rb_{qb}")
                nc.tensor.matmul(rb_ps, lhsT=ones_row, rhs=rsum,
                                 start=True, stop=True)
                rb_sb = rbp.tile([P, 512], f32, tag="rb_sb",
                                 name=f"rb_sb_{qb}")
                nc.vector.tensor_copy(out=rb_sb, in_=rb_ps)
                oav = [oavp.tile([P, 512], bf16, tag="oav",
                                 name=f"oav_{qb}_{co}") for co in range(NCT)]
                for co in range(NCT):
                    nc.vector.tensor_mul(out=oav[co], in0=av_ps[co],
                                         in1=rb_sb)
                for co in range(NCT):
                    fp_ps = psAV.tile([P, 512], f32, tag="av",
                                      name=f"fp_{qb}_{co}")
                    for ci in range(NCT):
                        nc.tensor.matmul(
                            fp_ps, lhsT=wo_sb[:, ci, co * P:(co + 1) * P],
                            rhs=oav[ci], start=(ci == 0),
                            stop=(ci == NCT - 1))
                    fout = foutp.tile([P, 512], f32, tag="fout",
                                      name=f"fout_{qb}_{co}")
                    nc.vector.tensor_scalar_add(out=fout, in0=fp_ps,
                                                scalar1=bias_o[:, co:co + 1])
                    nc.vector.tensor_add(out=fout, in0=fout,
                                         in1=x_own[:, co, qsl])
                    nc.sync.dma_start(out=out_d[co * P:(co + 1) * P, qsl],
                                      in_=fout)

    nc.finalize()
    return nc


def _get_nc():
    if "nc" not in _CACHE:
        _CACHE["nc"] = _build_bass()
    return _CACHE["nc"]


def _prepare_in_maps(inputs):
    x = np.asarray(inputs["x"], dtype=np.float32)
    b, c, t, h, w = x.shape
    assert (b, c, t, h * w) == (1, C, T, HW)

    wts = {}
    for name, key in (("wqT", "wq"), ("wkT", "wk"), ("wvT", "wv"),
                      ("woT", "wo")):
        wts[name] = np.ascontiguousarray(
            np.asarray(inputs[key], dtype=np.float32).T
        ).astype(ml_dtypes.bfloat16)
    params = np.stack([
        np.asarray(inputs["bq"]), np.asarray(inputs["bk"]),
        np.asarray(inputs["bv"]), np.asarray(inputs["bo"]),
        np.asarray(inputs["gamma"]), np.asarray(inputs["beta"]),
    ]).astype(np.float32)

    xr = np.ascontiguousarray(x.reshape(C, T, HW))
    in_maps = []
    for core in range(NCORES):
        f, half = core // 2, core % 2
        own = xr[:, f, half * HALF:(half + 1) * HALF]
        oth = xr[:, f, (1 - half) * HALF:(2 - half) * HALF]
        x_frame = np.ascontiguousarray(np.concatenate([own, oth], axis=1))
        in_maps.append({"x_frame": x_frame, "params": params, **wts})
    return in_maps


def _assemble_out(per_core_outs):
    out = np.empty((C, T, HW), np.float32)
    for core in range(NCORES):
        f, half = core // 2, core % 2
        out[:, f, half * HALF:(half + 1) * HALF] = per_core_outs[core]
    return out.reshape(1, C, T, 64, 64)


def kernel(**inputs):
    import os
    from concourse.bass_utils import run_bass_kernel_spmd

    in_maps = _prepare_in_maps(inputs)
    nc = _get_nc()
    trace = os.environ.get("KBENCH_TRACE") == "1"
    if trace:
        try:
            from antenv.axon_hooks import get_axon_ntff_profile_hook  # noqa: F401
        except ImportError:
            trace = False
    res = run_bass_kernel_spmd(nc, in_maps, core_ids=list(range(NCORES)),
                               trace=trace)
    _CACHE["last_res"] = res
    return _assemble_out([r["out"] for r in res.results])
